# revision 34
# baseline (speedup 1.0000x reference)
"""MoE layer (top-3-of-8 gating) on 8 Trainium2 NeuronCores.

Strategy: expert-parallel with host-side routing. The host computes the
gating softmax + top-3 in fp32, gathers each expert's routed tokens into
a compact slot array (NS = max_e ceil(n_e/128)*128 slots, ~1.04x the
ideal load), and pre-tiles the weights. Core c runs expert c's FFN over
its slots: h = relu(x@W1^T + b1) in bf16 with fp32 PSUM accumulation,
y = (h@W2^T) * w_gate fused into the PSUM->SBUF copy. The host combines
with 8 fancy-index adds (and folds in b2 exactly, if nonzero).

Self-contained: hardcodes M=8 cores; shapes B=8192, D=1024, H=4096,
E=8, K=3 come from the inputs.
"""

import os
import sys
from contextlib import ExitStack

sys.path.insert(0, "/opt/trn_rl_repo")

import ml_dtypes
import numpy as np

import concourse.bass as bass
import concourse.tile as tile
from concourse import bacc, mybir

P = 128
F32 = mybir.dt.float32
BF16 = mybir.dt.bfloat16
AF = mybir.ActivationFunctionType
ALU = mybir.AluOpType


def build_expert_ffn(nc, NS, D, H, SC=512, NSR=None):
    """Per-core Tile program: one expert's FFN over NS routed slots.

    DRAM inputs (per-core content, same shapes across cores):
      xt:  [P, ND*NS] bf16  — chunk-major x^T tiles; chunk c at cols
           [ND*c0, ND*(c0+sc)), within it d-major: [d*sc, (d+1)*sc)
      w1:  [NH, P, ND*P] bf16 — w1[i][dp, d*P+hh] = W1[e, i*P+hh, d*P+dp]
      w2:  [NH, P, D] bf16    — w2[i][hp, dc] = W2[e, dc, i*P+hp]
      b1:  [P, NH] f32        — b1[hp, i] = b1[e, i*P+hp]
      wv:  [P, NT] f32        — wv[p, t] = gate weight of slot t*P+p
      out: [NT, P, D] f32     — y[slot, :] scaled by gate weight
    """
    ND, NH, NT = D // P, H // P, NS // P
    # real (unpadded) slot count: mm1 streams only this many columns; the
    # rest of hT stays stale, and the zero gate weight nulls those slots
    NSR = NS if NSR is None else min(NS, NSR)
    # first chunk small so its x DMA (which gates the first matmul) is short
    chunks = []
    c0 = 0
    while c0 < NS:
        sc = min(SC // 2 if c0 == 0 else SC, NS - c0)
        chunks.append((c0, sc))
        c0 += sc

    xt_d = nc.declare_dram_parameter("xt", [P, ND * NS], BF16, isOutput=False)
    w1_d = nc.declare_dram_parameter("w1", [NH, P, ND * P], BF16, isOutput=False)
    w2_d = nc.declare_dram_parameter("w2", [NH, P, D], BF16, isOutput=False)
    b1_d = nc.declare_dram_parameter("b1", [P, NH], F32, isOutput=False)
    wv_d = nc.declare_dram_parameter("wv", [P, NT], F32, isOutput=False)
    out_d = nc.declare_dram_parameter("out", [NT, P, D], F32, isOutput=True)

    with ExitStack() as ctx:
        tc = ctx.enter_context(tile.TileContext(nc))
        const = ctx.enter_context(tc.tile_pool(name="const", bufs=1))
        w1p = ctx.enter_context(tc.tile_pool(name="w1p", bufs=NH))
        w2p = ctx.enter_context(tc.tile_pool(name="w2p", bufs=NH))
        xtp = ctx.enter_context(tc.tile_pool(name="xtp", bufs=2))
        hp = ctx.enter_context(tc.tile_pool(name="hp", bufs=1))
        outp = ctx.enter_context(tc.tile_pool(name="outp", bufs=3))
        php = ctx.enter_context(tc.tile_pool(name="php", bufs=3, space="PSUM"))
        pyp = ctx.enter_context(tc.tile_pool(name="pyp", bufs=3, space="PSUM"))

        def load_chunk(c0, sc, splits=1):
            # split across DMA queues so the load lands in ~sc*ND*2/(8*splits)
            # bytes per queue
            xt_sb = xtp.tile([P, ND * SC], BF16, tag="xt", name=f"xt_{c0}")
            ss = max(P, sc // splits)
            for d in range(ND):
                for s0 in range(0, sc, ss):
                    sw = min(ss, sc - s0)
                    nc.sync.dma_start(
                        xt_sb[:, d * sc + s0 : d * sc + s0 + sw],
                        xt_d[:, ND * c0 + d * sc + s0 : ND * c0 + d * sc + s0 + sw],
                    )
            return xt_sb

        # first x chunk ahead of the weight stream: it gates the first matmul
        xt_first = load_chunk(*chunks[0])
        b1_sb = const.tile([P, NH], F32, tag="b1")
        nc.sync.dma_start(b1_sb, b1_d[:])
        wv_sb = const.tile([P, NT], F32, tag="wv")
        nc.sync.dma_start(wv_sb, wv_d[:])
        w1t = []
        w2t = []
        for i in range(NH):
            w1ti = w1p.tile([P, ND * P], BF16, tag="w1", name=f"w1_{i}")
            if i < 2:
                # split the first tiles across queues: w1_0 gates matmul 0
                hw = ND * P // 2
                nc.sync.dma_start(w1ti[:, :hw], w1_d[i, :, :hw])
                nc.sync.dma_start(w1ti[:, hw:], w1_d[i, :, hw:])
            else:
                nc.sync.dma_start(w1ti, w1_d[i])
            w1t.append(w1ti)
        for i in range(NH):
            w2ti = w2p.tile([P, D], BF16, tag="w2", name=f"w2_{i}")
            nc.sync.dma_start(w2ti, w2_d[i])
            w2t.append(w2ti)

        hT = hp.tile([P, NH * SC], BF16, tag="hT")
        for ci, (c0, sc) in enumerate(chunks):
            se = max(0, min(sc, NSR - c0))  # real columns in this chunk
            if se == 0:
                continue
            xt_sb = xt_first if ci == 0 else load_chunk(c0, sc)
            # mm1: hT[i] = relu(W1 x^T + b1), bf16 out of fp32 PSUM
            for i in range(NH):
                ph = php.tile([P, SC], F32, tag="ph")
                for d in range(ND):
                    nc.tensor.matmul(
                        ph[:, :se],
                        lhsT=w1t[i][:, d * P : (d + 1) * P],
                        rhs=xt_sb[:, d * sc : d * sc + se],
                        start=(d == 0),
                        stop=(d == ND - 1),
                    )
                nc.scalar.activation(
                    hT[:, i * sc : i * sc + se],
                    ph[:, :se],
                    AF.Relu,
                    bias=b1_sb[:, i : i + 1],
                    scale=1.0,
                )
            # mm2: y[t] = (hT^T W2) * w_gate[t]; N=512 chunks of D per PSUM bank
            NC = min(512, D)
            NJ = D // NC
            for s in range(sc // P):
                t = c0 // P + s
                y = outp.tile([P, D], F32, tag="y")
                for j in range(NJ):
                    py = pyp.tile([P, NC], F32, tag="py")
                    for i in range(NH):
                        nc.tensor.matmul(
                            py,
                            lhsT=hT[:, i * sc + s * P : i * sc + (s + 1) * P],
                            rhs=w2t[i][:, j * NC : (j + 1) * NC],
                            start=(i == 0),
                            stop=(i == NH - 1),
                        )
                    nc.scalar.activation(
                        y[:, j * NC : (j + 1) * NC],
                        py,
                        AF.Copy,
                        scale=wv_sb[:, t : t + 1],
                    )
                    # per-half DMA (quartered on the final tile to cut the
                    # exposed tail): spreads queues and shortens the drain
                    nsplit = 2 if t == NT - 1 else 1
                    w = NC // nsplit
                    for q in range(nsplit):
                        lo = j * NC + q * w
                        nc.sync.dma_start(
                            out_d[t, :, lo : lo + w], y[:, lo : lo + w]
                        )
    return nc


def host_route(x, Wg, K=3):
    """Gating softmax + top-K on host, fp32. Returns (w_be, sel_idx)."""
    g = x.astype(np.float32) @ Wg.astype(np.float32).T  # [B, E]
    g = g - g.max(axis=1, keepdims=True)
    eg = np.exp(g)
    gating = eg / eg.sum(axis=1, keepdims=True)  # [B, E] fp32
    # stable descending argsort matches jax.lax.top_k tie-breaking
    idx = np.argsort(-gating, axis=1, kind="stable")[:, :K]  # [B, K]
    w_be = np.zeros_like(gating)
    rows = np.arange(gating.shape[0])[:, None]
    w_be[rows, idx] = gating[rows, idx]
    return w_be, idx


def host_prep(x, W1, b1, W2, w_be, M, SC=512):
    """Per-expert gather + weight pre-tiling. Returns (in_maps, meta)."""
    x = np.asarray(x, dtype=np.float32)
    W1 = np.asarray(W1, dtype=np.float32)
    b1 = np.asarray(b1, dtype=np.float32)
    W2 = np.asarray(W2, dtype=np.float32)
    B, D = x.shape
    E, H, _ = W1.shape
    ND, NH = D // P, H // P
    bf16 = ml_dtypes.bfloat16

    tok_idx = [np.nonzero(w_be[:, e])[0] for e in range(E)]
    n_e = [len(t) for t in tok_idx]
    NS = max(P, -(-max(n_e) // P) * P)
    NT = NS // P

    in_maps = []
    for e in range(E):
        xg = np.zeros((NS, D), dtype=np.float32)
        xg[: n_e[e]] = x[tok_idx[e]]
        # chunk-major x^T: [P, ND*NS], chunk c cols d-major (must mirror
        # the chunk structure in build_expert_ffn)
        blocks = []
        c0 = 0
        while c0 < NS:
            sc = min(SC // 2 if c0 == 0 else SC, NS - c0)
            blk = xg[c0 : c0 + sc].T.reshape(ND, P, sc).transpose(1, 0, 2)
            blocks.append(blk.reshape(P, ND * sc))
            c0 += sc
        xt = np.ascontiguousarray(np.concatenate(blocks, axis=1)).astype(bf16)

        w1x = np.ascontiguousarray(
            W1[e].reshape(NH, P, ND, P).transpose(0, 3, 2, 1).reshape(NH, P, ND * P)
        ).astype(bf16)
        w2x = np.ascontiguousarray(W2[e].T.reshape(NH, P, D)).astype(bf16)
        b1x = np.ascontiguousarray(b1[e].reshape(NH, P).T)

        wsl = np.zeros(NS, dtype=np.float32)
        wsl[: n_e[e]] = w_be[tok_idx[e], e]
        wvx = np.ascontiguousarray(wsl.reshape(NT, P).T)

        in_maps.append({"xt": xt, "w1": w1x, "w2": w2x, "b1": b1x, "wv": wvx})
    meta = dict(NS=NS, NSR=max(n_e), D=D, H=H, E=E, tok_idx=tok_idx, n_e=n_e)
    return in_maps, meta


def kernel(x, Wg, W1, b1, W2, b2):
    from concourse.bass_utils import run_bass_kernel_spmd

    M = 8
    x = np.asarray(x)
    B, D = x.shape
    E, H, _ = np.asarray(W1).shape
    assert E == M, (E, M)

    w_be, _ = host_route(x, Wg, K=3)
    in_maps, meta = host_prep(x, W1, b1, W2, w_be, M=M)

    nc = bacc.Bacc("TRN2", target_bir_lowering=False, debug=False, num_devices=M)
    build_expert_ffn(nc, meta["NS"], D, H, NSR=meta["NSR"])
    nc.finalize()

    trace = bool(os.environ.get("MOE_TRACE"))
    if trace:
        try:
            import hookshim

            hookshim.install()
        except Exception:
            pass
    res = run_bass_kernel_spmd(nc, in_maps, list(range(M)), trace=trace)
    if trace and res.exec_time_ns is not None:
        print(f"HW exec time: {res.exec_time_ns} ns")

    out = np.zeros((B, D), dtype=np.float32)
    for e in range(E):
        ye = res.results[e]["out"].reshape(meta["NS"], D)
        out[meta["tok_idx"][e]] += ye[: meta["n_e"][e]]
    b2 = np.asarray(b2, dtype=np.float32)
    if np.any(b2):
        out += w_be @ b2
    return out


# revision 35
# speedup vs baseline: 1.0051x; 1.0051x over previous
"""MoE layer (top-3-of-8 gating) on 8 Trainium2 NeuronCores.

Strategy: expert-parallel with host-side routing. The host computes the
gating softmax + top-3 in fp32, gathers each expert's routed tokens into
a compact slot array (NS = max_e ceil(n_e/128)*128 slots, ~1.04x the
ideal load), and pre-tiles the weights. Core c runs expert c's FFN over
its slots: h = relu(x@W1^T + b1) in bf16 with fp32 PSUM accumulation,
y = (h@W2^T) * w_gate fused into the PSUM->SBUF copy. The host combines
with 8 fancy-index adds (and folds in b2 exactly, if nonzero).

Self-contained: hardcodes M=8 cores; shapes B=8192, D=1024, H=4096,
E=8, K=3 come from the inputs.
"""

import os
import sys
from contextlib import ExitStack

sys.path.insert(0, "/opt/trn_rl_repo")

import ml_dtypes
import numpy as np

import concourse.bass as bass
import concourse.tile as tile
from concourse import bacc, mybir

P = 128
F32 = mybir.dt.float32
BF16 = mybir.dt.bfloat16
AF = mybir.ActivationFunctionType
ALU = mybir.AluOpType


def build_expert_ffn(nc, NS, D, H, SC=512, NSR=None):
    """Per-core Tile program: one expert's FFN over NS routed slots.

    DRAM inputs (per-core content, same shapes across cores):
      xt:  [P, ND*NS] bf16  — chunk-major x^T tiles; chunk c at cols
           [ND*c0, ND*(c0+sc)), within it d-major: [d*sc, (d+1)*sc)
      w1:  [NH, P, ND*P] bf16 — w1[i][dp, d*P+hh] = W1[e, i*P+hh, d*P+dp]
      w2:  [NH, P, D] bf16    — w2[i][hp, dc] = W2[e, dc, i*P+hp]
      b1:  [P, NH] f32        — b1[hp, i] = b1[e, i*P+hp]
      wv:  [P, NT] f32        — wv[p, t] = gate weight of slot t*P+p
      out: [NT, P, D] f32     — y[slot, :] scaled by gate weight
    """
    ND, NH, NT = D // P, H // P, NS // P
    # real (unpadded) slot count: mm1 streams only this many columns; the
    # rest of hT stays stale, and the zero gate weight nulls those slots
    NSR = NS if NSR is None else min(NS, NSR)
    # first chunk small so its x DMA (which gates the first matmul) is short
    chunks = []
    c0 = 0
    while c0 < NS:
        sc = min(SC // 2 if c0 == 0 else SC, NS - c0)
        chunks.append((c0, sc))
        c0 += sc

    xt_d = nc.declare_dram_parameter("xt", [P, ND * NS], BF16, isOutput=False)
    w1_d = nc.declare_dram_parameter("w1", [NH, P, ND * P], BF16, isOutput=False)
    w2_d = nc.declare_dram_parameter("w2", [NH, P, D], BF16, isOutput=False)
    b1_d = nc.declare_dram_parameter("b1", [P, NH], F32, isOutput=False)
    wv_d = nc.declare_dram_parameter("wv", [P, NT], F32, isOutput=False)
    out_d = nc.declare_dram_parameter("out", [NT, P, D], F32, isOutput=True)

    with ExitStack() as ctx:
        tc = ctx.enter_context(tile.TileContext(nc))
        const = ctx.enter_context(tc.tile_pool(name="const", bufs=1))
        w1p = ctx.enter_context(tc.tile_pool(name="w1p", bufs=NH))
        w2p = ctx.enter_context(tc.tile_pool(name="w2p", bufs=NH))
        xtp = ctx.enter_context(tc.tile_pool(name="xtp", bufs=2))
        hp = ctx.enter_context(tc.tile_pool(name="hp", bufs=1))
        outp = ctx.enter_context(tc.tile_pool(name="outp", bufs=3))
        php = ctx.enter_context(tc.tile_pool(name="php", bufs=4, space="PSUM"))
        pyp = ctx.enter_context(tc.tile_pool(name="pyp", bufs=4, space="PSUM"))

        def load_chunk(c0, sc, splits=1):
            # split across DMA queues so the load lands in ~sc*ND*2/(8*splits)
            # bytes per queue
            xt_sb = xtp.tile([P, ND * SC], BF16, tag="xt", name=f"xt_{c0}")
            ss = max(P, sc // splits)
            for d in range(ND):
                for s0 in range(0, sc, ss):
                    sw = min(ss, sc - s0)
                    nc.sync.dma_start(
                        xt_sb[:, d * sc + s0 : d * sc + s0 + sw],
                        xt_d[:, ND * c0 + d * sc + s0 : ND * c0 + d * sc + s0 + sw],
                    )
            return xt_sb

        # first x chunk ahead of the weight stream: it gates the first matmul
        xt_first = load_chunk(*chunks[0])
        b1_sb = const.tile([P, NH], F32, tag="b1")
        nc.sync.dma_start(b1_sb, b1_d[:])
        wv_sb = const.tile([P, NT], F32, tag="wv")
        nc.sync.dma_start(wv_sb, wv_d[:])
        w1t = []
        w2t = []
        for i in range(NH):
            w1ti = w1p.tile([P, ND * P], BF16, tag="w1", name=f"w1_{i}")
            if i < 2:
                # split the first tiles across queues: w1_0 gates matmul 0
                hw = ND * P // 2
                nc.sync.dma_start(w1ti[:, :hw], w1_d[i, :, :hw])
                nc.sync.dma_start(w1ti[:, hw:], w1_d[i, :, hw:])
            else:
                nc.sync.dma_start(w1ti, w1_d[i])
            w1t.append(w1ti)
        for i in range(NH):
            w2ti = w2p.tile([P, D], BF16, tag="w2", name=f"w2_{i}")
            nc.sync.dma_start(w2ti, w2_d[i])
            w2t.append(w2ti)

        hT = hp.tile([P, NH * SC], BF16, tag="hT")
        for ci, (c0, sc) in enumerate(chunks):
            se = max(0, min(sc, NSR - c0))  # real columns in this chunk
            if se == 0:
                continue
            xt_sb = xt_first if ci == 0 else load_chunk(c0, sc)
            # mm1: hT[i] = relu(W1 x^T + b1), bf16 out of fp32 PSUM
            for i in range(NH):
                ph = php.tile([P, SC], F32, tag="ph")
                for d in range(ND):
                    nc.tensor.matmul(
                        ph[:, :se],
                        lhsT=w1t[i][:, d * P : (d + 1) * P],
                        rhs=xt_sb[:, d * sc : d * sc + se],
                        start=(d == 0),
                        stop=(d == ND - 1),
                    )
                nc.scalar.activation(
                    hT[:, i * sc : i * sc + se],
                    ph[:, :se],
                    AF.Relu,
                    bias=b1_sb[:, i : i + 1],
                    scale=1.0,
                )
            # mm2: y[t] = (hT^T W2) * w_gate[t]; N=512 chunks of D per PSUM bank
            NC = min(512, D)
            NJ = D // NC
            for s in range(sc // P):
                t = c0 // P + s
                y = outp.tile([P, D], F32, tag="y")
                for j in range(NJ):
                    py = pyp.tile([P, NC], F32, tag="py")
                    for i in range(NH):
                        nc.tensor.matmul(
                            py,
                            lhsT=hT[:, i * sc + s * P : i * sc + (s + 1) * P],
                            rhs=w2t[i][:, j * NC : (j + 1) * NC],
                            start=(i == 0),
                            stop=(i == NH - 1),
                        )
                    nc.scalar.activation(
                        y[:, j * NC : (j + 1) * NC],
                        py,
                        AF.Copy,
                        scale=wv_sb[:, t : t + 1],
                    )
                    # per-half DMA (quartered on the final tile to cut the
                    # exposed tail): spreads queues and shortens the drain
                    nsplit = 2 if t == NT - 1 else 1
                    w = NC // nsplit
                    for q in range(nsplit):
                        lo = j * NC + q * w
                        nc.sync.dma_start(
                            out_d[t, :, lo : lo + w], y[:, lo : lo + w]
                        )
    return nc


def host_route(x, Wg, K=3):
    """Gating softmax + top-K on host, fp32. Returns (w_be, sel_idx)."""
    g = x.astype(np.float32) @ Wg.astype(np.float32).T  # [B, E]
    g = g - g.max(axis=1, keepdims=True)
    eg = np.exp(g)
    gating = eg / eg.sum(axis=1, keepdims=True)  # [B, E] fp32
    # stable descending argsort matches jax.lax.top_k tie-breaking
    idx = np.argsort(-gating, axis=1, kind="stable")[:, :K]  # [B, K]
    w_be = np.zeros_like(gating)
    rows = np.arange(gating.shape[0])[:, None]
    w_be[rows, idx] = gating[rows, idx]
    return w_be, idx


def host_prep(x, W1, b1, W2, w_be, M, SC=512):
    """Per-expert gather + weight pre-tiling. Returns (in_maps, meta)."""
    x = np.asarray(x, dtype=np.float32)
    W1 = np.asarray(W1, dtype=np.float32)
    b1 = np.asarray(b1, dtype=np.float32)
    W2 = np.asarray(W2, dtype=np.float32)
    B, D = x.shape
    E, H, _ = W1.shape
    ND, NH = D // P, H // P
    bf16 = ml_dtypes.bfloat16

    tok_idx = [np.nonzero(w_be[:, e])[0] for e in range(E)]
    n_e = [len(t) for t in tok_idx]
    NS = max(P, -(-max(n_e) // P) * P)
    NT = NS // P

    in_maps = []
    for e in range(E):
        xg = np.zeros((NS, D), dtype=np.float32)
        xg[: n_e[e]] = x[tok_idx[e]]
        # chunk-major x^T: [P, ND*NS], chunk c cols d-major (must mirror
        # the chunk structure in build_expert_ffn)
        blocks = []
        c0 = 0
        while c0 < NS:
            sc = min(SC // 2 if c0 == 0 else SC, NS - c0)
            blk = xg[c0 : c0 + sc].T.reshape(ND, P, sc).transpose(1, 0, 2)
            blocks.append(blk.reshape(P, ND * sc))
            c0 += sc
        xt = np.ascontiguousarray(np.concatenate(blocks, axis=1)).astype(bf16)

        w1x = np.ascontiguousarray(
            W1[e].reshape(NH, P, ND, P).transpose(0, 3, 2, 1).reshape(NH, P, ND * P)
        ).astype(bf16)
        w2x = np.ascontiguousarray(W2[e].T.reshape(NH, P, D)).astype(bf16)
        b1x = np.ascontiguousarray(b1[e].reshape(NH, P).T)

        wsl = np.zeros(NS, dtype=np.float32)
        wsl[: n_e[e]] = w_be[tok_idx[e], e]
        wvx = np.ascontiguousarray(wsl.reshape(NT, P).T)

        in_maps.append({"xt": xt, "w1": w1x, "w2": w2x, "b1": b1x, "wv": wvx})
    meta = dict(NS=NS, NSR=max(n_e), D=D, H=H, E=E, tok_idx=tok_idx, n_e=n_e)
    return in_maps, meta


def kernel(x, Wg, W1, b1, W2, b2):
    from concourse.bass_utils import run_bass_kernel_spmd

    M = 8
    x = np.asarray(x)
    B, D = x.shape
    E, H, _ = np.asarray(W1).shape
    assert E == M, (E, M)

    w_be, _ = host_route(x, Wg, K=3)
    in_maps, meta = host_prep(x, W1, b1, W2, w_be, M=M)

    nc = bacc.Bacc("TRN2", target_bir_lowering=False, debug=False, num_devices=M)
    build_expert_ffn(nc, meta["NS"], D, H, NSR=meta["NSR"])
    nc.finalize()

    trace = bool(os.environ.get("MOE_TRACE"))
    if trace:
        try:
            import hookshim

            hookshim.install()
        except Exception:
            pass
    res = run_bass_kernel_spmd(nc, in_maps, list(range(M)), trace=trace)
    if trace and res.exec_time_ns is not None:
        print(f"HW exec time: {res.exec_time_ns} ns")

    out = np.zeros((B, D), dtype=np.float32)
    for e in range(E):
        ye = res.results[e]["out"].reshape(meta["NS"], D)
        out[meta["tok_idx"][e]] += ye[: meta["n_e"][e]]
    b2 = np.asarray(b2, dtype=np.float32)
    if np.any(b2):
        out += w_be @ b2
    return out
